# revision 9
# baseline (speedup 1.0000x reference)
"""CARAFE content-aware upsampling kernel for Trainium2 (Bass/Tile), 8 NeuronCores.

Problem (hardcoded): features [4, 256, 64, 64] f32, masks [4, 25, 128, 128] f32,
K=5, G=1, S=2 -> output [4, 256, 128, 128] f32.

Strategy
--------
Sharding: 8 cores = (batch n in 0..3) x (output-row half yh in 0..1); each core
computes out[n, :, yh*64:(yh+1)*64, :] for ALL 256 channels. The banded mask
operand depends only on (n, y), so splitting y (not channels) halves its HBM
traffic.

Compute mapping: CARAFE's per-output-pixel 25-tap weighted sum is cast as
TensorEngine matmuls contracting over the padded input-width axis wp (K=68):

  out[c, 2*h2+py, x] = sum_{hp, wp} bnd[hp, wp, kr=hp-h2, py, x] * ft[wp, hp, c]

ft is the zero-padded transposed feature map (bf16); bnd is a host-built banded
mask operand (bf16): for input row hp and tap-row kr, column (py, x) holds
mask m[kr*5+dw, 2*(hp-kr)+py, x] at partition wp = floor(x/2)+dw, else zero.

Per local input row hpl (36 rows/core): 2 stationary loads (c-halves) and up to
10 matmuls ([68, 256] moving operand) accumulating f32 into the PSUM tile
[128, 512] of output pair h2 = hpl-kr (c-half 0 in cols 0:256, half 1 in
256:512). A rolling window of 5 PSUM tiles stays live; completed pairs DMA
straight from PSUM to HBM.
"""

import sys

sys.path.insert(0, "/opt/trn_rl_repo")

import numpy as np
import ml_dtypes

import concourse.bacc as bacc
import concourse.mybir as mybir
from concourse import tile
from concourse import bass_utils

N, C, H, W = 4, 256, 64, 64
S = 2
KK = 5
HO, WO = H * S, W * S  # 128, 128
HP = H + KK - 1  # 68 padded rows
WP = W + KK - 1  # 68 padded cols
NCORES = 8

HPL = 36  # padded input rows per core (32 pairs + 4 tap overlap)
NPAIR = 32  # output row-pairs per core
NBLK = 9  # hpl DMA blocks of 4
FW = KK * 2 * WO  # 1280 band cols per input row

BF16 = ml_dtypes.bfloat16


def _host_prep(features: np.ndarray, masks: np.ndarray):
    """Per-core transposed/padded features and banded mask operands."""
    # ft_g[n, wp, hp, c] = features[n, c, hp-2, wp-2]  (zero pad)
    ft_g = np.zeros((N, WP, HP, C), np.float32)
    ft_g[:, 2 : 2 + W, 2 : 2 + H, :] = features.transpose(0, 3, 2, 1)
    ft_g = ft_g.astype(BF16)

    # bnd_g[n, hp, wp, kr, py, x] = masks[n, kr*5+dw, 2*(hp-kr)+py, x]
    #   where dw = wp - floor(x/2), nonzero only for dw in [0, 5)
    bnd_g = np.zeros((N, HP, WP, KK, 2, WO), np.float32)
    st = [s // 4 for s in bnd_g.strides]  # element strides
    m6 = masks.reshape(N, KK * KK, H, 2, W, 2)
    for kr in range(KK):
        for dw in range(KK):
            base = bnd_g[:, kr:, dw:, kr, :, :]
            view = np.lib.stride_tricks.as_strided(
                base,
                shape=(N, H, 2, W, 2),
                strides=tuple(
                    4 * s
                    for s in (st[0], st[1], st[4], st[2] + 2 * st[5], st[5])
                ),
            )
            view[...] = m6[:, kr * KK + dw]
    bnd_g = bnd_g.astype(BF16)

    fts, bnds = [], []
    for i in range(NCORES):
        n, yh = divmod(i, 2)
        fts.append(np.ascontiguousarray(ft_g[n, :, yh * NPAIR : yh * NPAIR + HPL, :]))
        b = bnd_g[n, yh * NPAIR : yh * NPAIR + HPL].reshape(NBLK, 4, WP, FW)
        bnds.append(np.ascontiguousarray(b.transpose(0, 2, 1, 3)))
    return fts, bnds


_NC_CACHE = []


def _build_nc():
    """Build + compile the single-core Tile program (same for all 8 cores)."""
    if _NC_CACHE:
        return _NC_CACHE[0]

    nc = bacc.Bacc("TRN2", target_bir_lowering=False, debug=False)
    ft = nc.dram_tensor(
        "ft", [WP, HPL * C], mybir.dt.bfloat16, kind="ExternalInput"
    ).ap()
    bnd = nc.dram_tensor(
        "bnd", [NBLK, WP, 4 * FW], mybir.dt.bfloat16, kind="ExternalInput"
    ).ap()
    out = nc.dram_tensor(
        "out", [C, 2 * NPAIR, WO], mybir.dt.float32, kind="ExternalOutput"
    ).ap()
    outf = out.rearrange("c y x -> c (y x)")  # [256, 64*128]

    with tile.TileContext(nc) as tc:
        with (
            tc.tile_pool(name="ftp", bufs=1) as ftp,
            tc.tile_pool(name="bnp", bufs=3) as bnp,
            tc.tile_pool(name="pp", bufs=7, space="PSUM") as pp,
            tc.tile_pool(name="op", bufs=3) as op,
        ):
            ft_all = ftp.tile([WP, HPL * C], mybir.dt.bfloat16)
            nc.sync.dma_start(ft_all[:], ft)
            psums = {}
            for blk in range(NBLK):
                bnt = bnp.tile([WP, 4 * FW], mybir.dt.bfloat16)
                nc.sync.dma_start(bnt[:], bnd[blk])
                for i4 in range(4):
                    hpl = 4 * blk + i4
                    for ch in (0, 1):
                        lhsT = ft_all[:, hpl * C + ch * 128 : hpl * C + ch * 128 + 128]
                        for kr in range(KK):
                            h2 = hpl - kr
                            if not (0 <= h2 < NPAIR):
                                continue
                            if kr == 0 and ch == 0:
                                psums[h2] = pp.tile(
                                    [128, 2 * 2 * WO], mybir.dt.float32,
                                    name="ps", tag="ps",
                                )
                            # One PSUM accumulation group per pair tile (zero
                            # regions are bank-granular): open at the first
                            # matmul (ch0/kr0), close at the last (ch1/kr4).
                            nc.tensor.matmul(
                                psums[h2][:, ch * 2 * WO : (ch + 1) * 2 * WO],
                                lhsT,
                                bnt[:, i4 * FW + kr * 2 * WO : i4 * FW + (kr + 1) * 2 * WO],
                                start=(kr == 0 and ch == 0),
                                stop=(kr == KK - 1 and ch == 1),
                            )
                    h2 = hpl - (KK - 1)
                    if 0 <= h2 < NPAIR:
                        pt = psums.pop(h2)
                        g = h2 % 4
                        if g == 0:
                            ot = op.tile([128, 4 * 512], mybir.dt.float32,
                                         name="ot", tag="ot")
                            psums["ot"] = ot
                        ot = psums["ot"]
                        # staging cols: [ch, g, py*x] to keep DMA APs 3-dim
                        otv = ot.rearrange("p (ch g f) -> p ch g f", ch=2, g=4)
                        nc.vector.tensor_copy(
                            otv[:, :, g, :],
                            pt.rearrange("p (ch f) -> p ch f", ch=2),
                        )
                        if g == 3:
                            sv = ot.rearrange("p (ch gf) -> p ch gf", ch=2)
                            ov = outf.rearrange("(ch p) f -> p ch f", ch=2)
                            g0 = h2 - 3
                            nc.sync.dma_start(
                                ov[:, :, 2 * WO * g0 : 2 * WO * (g0 + 4)], sv
                            )

    nc.compile()
    _NC_CACHE.append(nc)
    return nc


def kernel(features: np.ndarray, masks: np.ndarray) -> np.ndarray:
    features = np.ascontiguousarray(features, dtype=np.float32)
    masks = np.ascontiguousarray(masks, dtype=np.float32)
    fts, bnds = _host_prep(features, masks)

    nc = _build_nc()
    in_maps = [
        {"ft": fts[i].reshape(WP, HPL * C), "bnd": bnds[i].reshape(NBLK, WP, 4 * FW)}
        for i in range(NCORES)
    ]

    res = bass_utils.run_bass_kernel_spmd(nc, in_maps, list(range(NCORES)))

    out = np.empty((N, C, HO, WO), np.float32)
    for i in range(NCORES):
        n, yh = divmod(i, 2)
        out[n, :, yh * 2 * NPAIR : (yh + 1) * 2 * NPAIR, :] = res.results[i][
            "out"
        ].reshape(C, 2 * NPAIR, WO)
    return out


# revision 18
# speedup vs baseline: 1.2866x; 1.2866x over previous
"""CARAFE content-aware upsampling kernel for Trainium2 (Bass/Tile), 8 NeuronCores.

Problem (hardcoded): features [4, 256, 64, 64] f32, masks [4, 25, 128, 128] f32,
K=5, G=1, S=2 -> output [4, 256, 128, 128] f32.

Strategy
--------
Sharding: 8 cores = (batch n in 0..3) x (output-row half yh in 0..1); each core
computes out[n, :, yh*64:(yh+1)*64, :] for ALL 256 channels. The banded mask
operand depends only on (n, y), so splitting y (not channels) halves its HBM
traffic.

Compute mapping: CARAFE's per-output-pixel 25-tap weighted sum is cast as
TensorEngine matmuls contracting over the padded input-width axis wp (K=68):

  out[c, 2*h2+py, x] = sum_{hp, wp} bnd[hp, wp, kr=hp-h2, py, x] * ft[wp, hp, c]

ft is the zero-padded transposed feature map (bf16); bnd is a host-built banded
mask operand (bf16): for input row hp and tap-row kr, column (py, x) holds
mask m[kr*5+dw, 2*(hp-kr)+py, x] at partition wp = floor(x/2)+dw, else zero.

Per local input row hpl (36 rows/core): 2 stationary loads (c-halves) and up to
10 matmuls ([68, 256] moving operand) accumulating f32 into the PSUM tile
[128, 512] of output pair h2 = hpl-kr (c-half 0 in cols 0:256, half 1 in
256:512). A rolling window of 5 PSUM tiles stays live; completed pairs DMA
straight from PSUM to HBM.
"""

import sys

sys.path.insert(0, "/opt/trn_rl_repo")

import numpy as np
import ml_dtypes

import concourse.bacc as bacc
import concourse.mybir as mybir
from concourse import tile
from concourse import bass_utils

N, C, H, W = 4, 256, 64, 64
S = 2
KK = 5
HO, WO = H * S, W * S  # 128, 128
HP = H + KK - 1  # 68 padded rows
WP = W + KK - 1  # 68 padded cols
NCORES = 8

HPL = 36  # padded input rows per core (32 pairs + 4 tap overlap)
NPAIR = 32  # output row-pairs per core
NBLK = 18  # hpl DMA blocks of 2
BLKH = 2  # hpl rows per band DMA block
FW = KK * 2 * WO  # 1280 band cols per input row

BF16 = ml_dtypes.bfloat16


def _host_prep(features: np.ndarray, masks: np.ndarray):
    """Per-core transposed/padded features and banded mask operands."""
    # ft_g[n, wp, hp, c] = features[n, c, hp-2, wp-2]  (zero pad)
    ft_g = np.zeros((N, WP, HP, C), np.float32)
    ft_g[:, 2 : 2 + W, 2 : 2 + H, :] = features.transpose(0, 3, 2, 1)
    ft_g = ft_g.astype(BF16)

    # bnd_g[n, hp, wp, kr, py, x] = masks[n, kr*5+dw, 2*(hp-kr)+py, x]
    #   where dw = wp - floor(x/2), nonzero only for dw in [0, 5)
    bnd_g = np.zeros((N, HP, WP, KK, 2, WO), np.float32)
    st = [s // 4 for s in bnd_g.strides]  # element strides
    m6 = masks.reshape(N, KK * KK, H, 2, W, 2)
    for kr in range(KK):
        for dw in range(KK):
            base = bnd_g[:, kr:, dw:, kr, :, :]
            view = np.lib.stride_tricks.as_strided(
                base,
                shape=(N, H, 2, W, 2),
                strides=tuple(
                    4 * s
                    for s in (st[0], st[1], st[4], st[2] + 2 * st[5], st[5])
                ),
            )
            view[...] = m6[:, kr * KK + dw]
    bnd_g = bnd_g.astype(BF16)

    fts, bnds = [], []
    for i in range(NCORES):
        n, yh = divmod(i, 2)
        fts.append(np.ascontiguousarray(ft_g[n, :, yh * NPAIR : yh * NPAIR + HPL, :]))
        b = bnd_g[n, yh * NPAIR : yh * NPAIR + HPL].reshape(NBLK, BLKH, WP, FW)
        bnds.append(np.ascontiguousarray(b.transpose(0, 2, 1, 3)))
    return fts, bnds


_NC_CACHE = []


def _build_nc():
    """Build + compile the single-core Tile program (same for all 8 cores)."""
    if _NC_CACHE:
        return _NC_CACHE[0]

    nc = bacc.Bacc("TRN2", target_bir_lowering=False, debug=False)
    ft = nc.dram_tensor(
        "ft", [WP, HPL * C], mybir.dt.bfloat16, kind="ExternalInput"
    ).ap()
    bnd = nc.dram_tensor(
        "bnd", [NBLK, WP, BLKH * FW], mybir.dt.bfloat16, kind="ExternalInput"
    ).ap()
    out = nc.dram_tensor(
        "out", [C, 2 * NPAIR, WO], mybir.dt.bfloat16, kind="ExternalOutput"
    ).ap()
    outf = out.rearrange("c y x -> c (y x)")  # [256, 64*128]

    with tile.TileContext(nc) as tc:
        with (
            tc.tile_pool(name="ftp", bufs=4) as ftp,
            tc.tile_pool(name="bnp", bufs=4) as bnp,
            tc.tile_pool(name="pp", bufs=7, space="PSUM") as pp,
            tc.tile_pool(name="op", bufs=3) as op,
        ):
            psums = {}
            ft_tiles = {}
            FC = 9  # hpl rows per feature chunk tile
            for blk in range(NBLK):
                bnt = bnp.tile([WP, BLKH * FW], mybir.dt.bfloat16)
                nc.sync.dma_start(bnt[:], bnd[blk])
                if blk in (0, 2, 4, 6):
                    # interleave feature chunks between the band blocks
                    ci = blk // 2
                    fct = ftp.tile([WP, FC * C], mybir.dt.bfloat16,
                                   name="fct", tag="fct")
                    nc.sync.dma_start(
                        fct[:], ft[:, ci * FC * C : (ci + 1) * FC * C]
                    )
                    ft_tiles[ci] = fct
                for i4 in range(BLKH):
                    hpl = BLKH * blk + i4
                    for ch in (0, 1):
                        fci, fcr = divmod(hpl, FC)
                        lhsT = ft_tiles[fci][
                            :, fcr * C + ch * 128 : fcr * C + ch * 128 + 128
                        ]
                        for kr in range(KK):
                            h2 = hpl - kr
                            if not (0 <= h2 < NPAIR):
                                continue
                            if kr == 0 and ch == 0:
                                psums[h2] = pp.tile(
                                    [128, 2 * 2 * WO], mybir.dt.float32,
                                    name="ps", tag="ps",
                                )
                            # One PSUM accumulation group per pair tile (zero
                            # regions are bank-granular): open at the first
                            # matmul (ch0/kr0), close at the last (ch1/kr4).
                            nc.tensor.matmul(
                                psums[h2][:, ch * 2 * WO : (ch + 1) * 2 * WO],
                                lhsT,
                                bnt[:, i4 * FW + kr * 2 * WO : i4 * FW + (kr + 1) * 2 * WO],
                                start=(kr == 0 and ch == 0),
                                stop=(kr == KK - 1 and ch == 1),
                            )
                    h2 = hpl - (KK - 1)
                    if 0 <= h2 < NPAIR:
                        pt = psums.pop(h2)
                        g = h2 % 2
                        if g == 0:
                            ot = op.tile([128, 2 * 512], mybir.dt.bfloat16,
                                         name="ot", tag="ot")
                            psums["ot"] = ot
                        ot = psums["ot"]
                        # staging cols: [ch, g, py*x] to keep DMA APs 3-dim
                        otv = ot.rearrange("p (ch g f) -> p ch g f", ch=2, g=2)
                        src = pt.rearrange("p (ch f) -> p ch f", ch=2)
                        if (h2 // 2) % 2 == 0:
                            nc.vector.tensor_copy(otv[:, :, g, :], src)
                        else:
                            nc.scalar.copy(otv[:, :, g, :], src)
                        if g == 1:
                            sv = ot.rearrange("p (ch gf) -> p ch gf", ch=2)
                            ov = outf.rearrange("(ch p) f -> p ch f", ch=2)
                            g0 = h2 - 1
                            nc.sync.dma_start(
                                ov[:, :, 2 * WO * g0 : 2 * WO * (g0 + 2)], sv
                            )

    nc.compile()
    _NC_CACHE.append(nc)
    return nc


def kernel(features: np.ndarray, masks: np.ndarray) -> np.ndarray:
    features = np.ascontiguousarray(features, dtype=np.float32)
    masks = np.ascontiguousarray(masks, dtype=np.float32)
    fts, bnds = _host_prep(features, masks)

    nc = _build_nc()
    in_maps = [
        {"ft": fts[i].reshape(WP, HPL * C), "bnd": bnds[i].reshape(NBLK, WP, BLKH * FW)}
        for i in range(NCORES)
    ]

    res = bass_utils.run_bass_kernel_spmd(nc, in_maps, list(range(NCORES)))

    out = np.empty((N, C, HO, WO), np.float32)
    for i in range(NCORES):
        n, yh = divmod(i, 2)
        out[n, :, yh * 2 * NPAIR : (yh + 1) * 2 * NPAIR, :] = (
            res.results[i]["out"].astype(np.float32).reshape(C, 2 * NPAIR, WO)
        )
    return out


# revision 21
# speedup vs baseline: 1.4091x; 1.0952x over previous
"""CARAFE content-aware upsampling kernel for Trainium2 (Bass/Tile), 8 NeuronCores.

Problem (hardcoded): features [4, 256, 64, 64] f32, masks [4, 25, 128, 128] f32,
K=5, G=1, S=2 -> output [4, 256, 128, 128] f32.

Strategy
--------
Sharding: 8 cores = (batch n in 0..3) x (output-row half yh in 0..1); each core
computes out[n, :, yh*64:(yh+1)*64, :] for ALL 256 channels. The banded mask
operand depends only on (n, y), so splitting y (not channels) halves its HBM
traffic.

Compute mapping: CARAFE's per-output-pixel 25-tap weighted sum is cast as
TensorEngine matmuls contracting over the padded input-width axis wp (K=68):

  out[c, 2*h2+py, x] = sum_{hp, wp} bnd[hp, wp, kr=hp-h2, py, x] * ft[wp, hp, c]

ft is the zero-padded transposed feature map (bf16); bnd is a host-built banded
mask operand (bf16): for input row hp and tap-row kr, column (py, x) holds
mask m[kr*5+dw, 2*(hp-kr)+py, x] at partition wp = floor(x/2)+dw, else zero.

Per local input row hpl (36 rows/core): 2 stationary loads (c-halves) and up to
10 matmuls ([68, 256] moving operand) accumulating f32 into the PSUM tile
[128, 512] of output pair h2 = hpl-kr (c-half 0 in cols 0:256, half 1 in
256:512). A rolling window of 5 PSUM tiles stays live; completed pairs DMA
straight from PSUM to HBM.
"""

import sys

sys.path.insert(0, "/opt/trn_rl_repo")

import numpy as np
import ml_dtypes

import concourse.bacc as bacc
import concourse.mybir as mybir
from concourse import tile
from concourse import bass_utils

N, C, H, W = 4, 256, 64, 64
S = 2
KK = 5
HO, WO = H * S, W * S  # 128, 128
HP = H + KK - 1  # 68 padded rows
WP = W + KK - 1  # 68 padded cols
NCORES = 8

HPL = 36  # padded input rows per core (32 pairs + 4 tap overlap)
NPAIR = 32  # output row-pairs per core
NBLK = 18  # hpl DMA blocks of 2
BLKH = 2  # hpl rows per band DMA block
FW = KK * 2 * WO  # 1280 band cols per input row

BF16 = ml_dtypes.bfloat16


def _host_prep(features: np.ndarray, masks: np.ndarray):
    """Per-core transposed/padded features and banded mask operands."""
    # ft_g[n, wp, hp, c] = features[n, c, hp-2, wp-2]  (zero pad)
    ft_g = np.zeros((N, WP, HP, C), np.float32)
    ft_g[:, 2 : 2 + W, 2 : 2 + H, :] = features.transpose(0, 3, 2, 1)
    ft_g = ft_g.astype(BF16)

    # bnd_g[n, hp, wp, kr, py, x] = masks[n, kr*5+dw, 2*(hp-kr)+py, x]
    #   where dw = wp - floor(x/2), nonzero only for dw in [0, 5)
    bnd_g = np.zeros((N, HP, WP, KK, 2, WO), np.float32)
    st = [s // 4 for s in bnd_g.strides]  # element strides
    m6 = masks.reshape(N, KK * KK, H, 2, W, 2)
    for kr in range(KK):
        for dw in range(KK):
            base = bnd_g[:, kr:, dw:, kr, :, :]
            view = np.lib.stride_tricks.as_strided(
                base,
                shape=(N, H, 2, W, 2),
                strides=tuple(
                    4 * s
                    for s in (st[0], st[1], st[4], st[2] + 2 * st[5], st[5])
                ),
            )
            view[...] = m6[:, kr * KK + dw]
    bnd_g = bnd_g.astype(BF16)

    fts, bnds = [], []
    for i in range(NCORES):
        n, yh = divmod(i, 2)
        fts.append(np.ascontiguousarray(ft_g[n, :, yh * NPAIR : yh * NPAIR + HPL, :]))
        b = bnd_g[n, yh * NPAIR : yh * NPAIR + HPL].reshape(NBLK, BLKH, WP, FW)
        bnds.append(np.ascontiguousarray(b.transpose(0, 2, 1, 3)))
    return fts, bnds


_NC_CACHE = []


def _build_nc():
    """Build + compile the single-core Tile program (same for all 8 cores)."""
    if _NC_CACHE:
        return _NC_CACHE[0]

    nc = bacc.Bacc("TRN2", target_bir_lowering=False, debug=False)
    ft = nc.dram_tensor(
        "ft", [WP, HPL * C], mybir.dt.bfloat16, kind="ExternalInput"
    ).ap()
    bnd = nc.dram_tensor(
        "bnd", [NBLK, WP, BLKH * FW], mybir.dt.bfloat16, kind="ExternalInput"
    ).ap()
    out = nc.dram_tensor(
        "out", [C, 2 * NPAIR, WO], mybir.dt.bfloat16, kind="ExternalOutput"
    ).ap()
    outf = out.rearrange("c y x -> c (y x)")  # [256, 64*128]

    with tile.TileContext(nc) as tc:
        with (
            tc.tile_pool(name="ftp", bufs=4) as ftp,
            tc.tile_pool(name="bnp", bufs=6) as bnp,
            tc.tile_pool(name="pp", bufs=8, space="PSUM") as pp,
            tc.tile_pool(name="op", bufs=4) as op,
        ):
            psums = {}
            ft_tiles = {}
            FC = 9  # hpl rows per feature chunk tile
            for blk in range(NBLK):
                bnt = bnp.tile([WP, BLKH * FW], mybir.dt.bfloat16)
                nc.sync.dma_start(bnt[:], bnd[blk])
                if blk in (0, 2, 4, 6):
                    # interleave feature chunks between the band blocks
                    ci = blk // 2
                    fct = ftp.tile([WP, FC * C], mybir.dt.bfloat16,
                                   name="fct", tag="fct")
                    nc.scalar.dma_start(
                        fct[:], ft[:, ci * FC * C : (ci + 1) * FC * C]
                    )
                    ft_tiles[ci] = fct
                for i4 in range(BLKH):
                    hpl = BLKH * blk + i4
                    for ch in (0, 1):
                        fci, fcr = divmod(hpl, FC)
                        lhsT = ft_tiles[fci][
                            :, fcr * C + ch * 128 : fcr * C + ch * 128 + 128
                        ]
                        for kr in range(KK):
                            h2 = hpl - kr
                            if not (0 <= h2 < NPAIR):
                                continue
                            if kr == 0 and ch == 0:
                                psums[h2] = pp.tile(
                                    [128, 2 * 2 * WO], mybir.dt.float32,
                                    name="ps", tag="ps",
                                )
                            # One PSUM accumulation group per pair tile (zero
                            # regions are bank-granular): open at the first
                            # matmul (ch0/kr0), close at the last (ch1/kr4).
                            nc.tensor.matmul(
                                psums[h2][:, ch * 2 * WO : (ch + 1) * 2 * WO],
                                lhsT,
                                bnt[:, i4 * FW + kr * 2 * WO : i4 * FW + (kr + 1) * 2 * WO],
                                start=(kr == 0 and ch == 0),
                                stop=(kr == KK - 1 and ch == 1),
                            )
                    h2 = hpl - (KK - 1)
                    if 0 <= h2 < NPAIR:
                        pt = psums.pop(h2)
                        g = h2 % 2
                        if g == 0:
                            ot = op.tile([128, 2 * 512], mybir.dt.bfloat16,
                                         name="ot", tag="ot")
                            psums["ot"] = ot
                        ot = psums["ot"]
                        # staging cols: [ch, g, py*x] to keep DMA APs 3-dim
                        otv = ot.rearrange("p (ch g f) -> p ch g f", ch=2, g=2)
                        src = pt.rearrange("p (ch f) -> p ch f", ch=2)
                        if (h2 // 2) % 2 == 0:
                            nc.vector.tensor_copy(otv[:, :, g, :], src)
                        else:
                            nc.scalar.copy(otv[:, :, g, :], src)
                        if g == 1:
                            sv = ot.rearrange("p (ch gf) -> p ch gf", ch=2)
                            ov = outf.rearrange("(ch p) f -> p ch f", ch=2)
                            g0 = h2 - 1
                            deng = nc.scalar if (h2 // 2) % 2 == 0 else nc.sync
                            deng.dma_start(
                                ov[:, :, 2 * WO * g0 : 2 * WO * (g0 + 2)], sv
                            )

    nc.compile()
    _NC_CACHE.append(nc)
    return nc


def kernel(features: np.ndarray, masks: np.ndarray) -> np.ndarray:
    features = np.ascontiguousarray(features, dtype=np.float32)
    masks = np.ascontiguousarray(masks, dtype=np.float32)
    fts, bnds = _host_prep(features, masks)

    nc = _build_nc()
    in_maps = [
        {"ft": fts[i].reshape(WP, HPL * C), "bnd": bnds[i].reshape(NBLK, WP, BLKH * FW)}
        for i in range(NCORES)
    ]

    res = bass_utils.run_bass_kernel_spmd(nc, in_maps, list(range(NCORES)))

    out = np.empty((N, C, HO, WO), np.float32)
    for i in range(NCORES):
        n, yh = divmod(i, 2)
        out[n, :, yh * 2 * NPAIR : (yh + 1) * 2 * NPAIR, :] = (
            res.results[i]["out"].astype(np.float32).reshape(C, 2 * NPAIR, WO)
        )
    return out
